# revision 9
# baseline (speedup 1.0000x reference)
"""Trainium2 Bass kernel for the GRU decoder (nn_Decoder_13168369730058).

Math (from the reference):
  h0 = encoder_outputs[0, :, -1, :]                       # (128, 512)
  step 1:   h1 = gru_cell(x=0, h0)
  step t>1: h_t = gru_cell(h_{t-1}, h_{t-1})   (carry is (h_new, h_new))

Because x == h from step 2 on, the two GRU matmuls fuse per gate:
  r  = sigmoid(Wr h + br)          Wr = Wih_r + Whh_r,  br = bih_r + bhh_r
  z  = sigmoid(Wz h + bz)          Wz = Wih_z + Whh_z,  bz = bih_z + bhh_z
  n  = tanh(Win h + bin + r * (Whn h + bhn))
  h' = n + z * (h - n)
Step 1 is the same with Wr,Wz -> Whh_{r,z} and no Win matmul (x = 0).

Distribution: data-parallel over batch, 16 rows per core on 8 cores, weights
replicated; the out_len recurrence is local to each core.

Layout: fully transposed on chip (H on partitions, batch on free dim). Each
128x128 weight block is the stationary operand (fp16 -> fast weight load;
LDW+MM pairs sustain ~27ns in a dense stream) and the transposed hidden
state h^T is the moving operand, producing gate pre-activations in PSUM.

The recurrence's serial tail (sigmoid_z -> z*(h-n) -> +n, ~0.8us with
cross-engine semaphore latencies) cannot be hidden inside a single
recurrence's own matmul stream, so each core runs TWO independent 8-row
batch groups, interleaved: while group A's tail executes on ACT/DVE,
group B's 69-pair matmul stream keeps the PE busy, and vice versa. Steady
state is a continuous PE stream; weight loads run twice per global step
(once per group), which is the FWL-bandwidth price of full overlap.

Per group and step, in PE order:
  [bias_r][r x16][bias_hn][hn x16][bias_in][in x16][idMM][bias_z][z x16]
  - biases are seeded into each gate's PSUM tile by tiny stationary
    matmuls, interleaved so each gate's bias directly precedes its block
    (keeps sigma_r as early as possible);
  - r*hn is injected into the in-gate PSUM by an identity matmul so tanh
    reads a complete pre-activation straight from PSUM;
  - the z-block is forced (add_dep_helper) after the idMM, and sigma_z
    after tanh, so the scheduler keeps the tail minimal.
State, tail tensors, and the DMA'd output are all fp16 (validated 6.2e-4
rel err vs the fp32 reference; the trajectory is contractive, not chaotic).
"""

import os
import numpy as np

import concourse.bacc as bacc
import concourse.mybir as mybir
import concourse.tile as tile
from concourse.tile import add_dep_helper
from concourse.bass_utils import run_bass_kernel_spmd

H = 512
BATCH = 128
N_CORES = int(os.environ.get("GRU_N_CORES", "8"))
T_STEPS = int(os.environ.get("GRU_T_STEPS", "1024"))
B_LOC = BATCH // N_CORES  # local batch per core (16)
GROUPS = 2                # independent recurrence groups per core
BG = B_LOC // GROUPS      # batch rows per group (8)
KT = H // 128             # 4 k-tiles

F32 = mybir.dt.float32
F16 = mybir.dt.float16


def _build(T: int):
    """Build the Bass program: T steps, GROUPS interleaved groups of BG rows."""
    nc = bacc.Bacc()
    b = BG

    # wc blocks: [r x16, hn x16, in x16, z x16]; each gate m-tile-major, then k
    wc_d = nc.dram_tensor("wc", [128, 64 * 128], F16, kind="ExternalInput")
    # w1 blocks: [r x16, hn x16, z x16] (step 1, weights = W_hh only)
    w1_d = nc.dram_tensor("w1", [128, 48 * 128], F16, kind="ExternalInput")
    # bias stationaries: rows 0-3 b_r, 4-7 b_hn, 8-11 b_in, 12-15 b_z
    bst_d = nc.dram_tensor("bst", [16, 128], F16, kind="ExternalInput")
    ones4_d = nc.dram_tensor("ones4", [4, 4 * b], F16, kind="ExternalInput")
    ident_d = nc.dram_tensor("ident", [128, 128], F16, kind="ExternalInput")
    h0_d = nc.dram_tensor("h0t", [128, GROUPS * 4 * b], F16,
                          kind="ExternalInput")
    out_d = nc.dram_tensor("outT", [T, 128, GROUPS * 4 * b], F16,
                           kind="ExternalOutput")

    sig = mybir.ActivationFunctionType.Sigmoid
    tanh = mybir.ActivationFunctionType.Tanh

    with tile.TileContext(nc) as tc:
        with (
            tc.tile_pool(name="singles", bufs=1) as singles,
            tc.tile_pool(name="state", bufs=2) as state,
            tc.tile_pool(name="work", bufs=2) as work,
            tc.tile_pool(name="psum", bufs=1, space="PSUM") as psum,
        ):
            w_sb = singles.tile([128, 64 * 128], F16)
            nc.sync.dma_start(w_sb[:], wc_d[:])
            w1_sb = singles.tile([128, 48 * 128], F16)
            nc.sync.dma_start(w1_sb[:], w1_d[:])
            bias_sb = {}
            for gi, gname in enumerate(("r", "hn", "in", "z")):
                t_ = singles.tile([4, 128], F16, tag=f"b{gname}")
                nc.sync.dma_start(t_[:], bst_d[4 * gi : 4 * (gi + 1)])
                bias_sb[gname] = t_
            ones4_sb = singles.tile([4, 4 * b], F16)
            nc.sync.dma_start(ones4_sb[:], ones4_d[:])
            ident_sb = singles.tile([128, 128], F16)
            nc.sync.dma_start(ident_sb[:], ident_d[:])

            h16 = []
            for g in range(GROUPS):
                hg = state.tile([128, 4 * b], F16, tag=f"h16_{g}")
                nc.sync.dma_start(hg[:], h0_d[:, g * 4 * b : (g + 1) * 4 * b])
                h16.append(hg)

            # Warm-up: hardware allows ONE embedded sync wait per instruction;
            # have the PE observe every init DMA here so loop matmuls carry a
            # single cross-engine wait (on the group's h16 only).
            warm_ps = psum.tile([128, 8], F32, tag="ps_r0")
            nc.tensor.matmul(warm_ps[:, 0:8], w_sb[:, 0:128], w_sb[:, 0:8],
                             start=True, stop=True)
            nc.tensor.matmul(warm_ps[:, 0:8], w1_sb[:, 0:128], w1_sb[:, 0:8],
                             start=True, stop=True)
            nc.tensor.matmul(warm_ps[:, 0:8], ident_sb[:], w_sb[:, 0:8],
                             start=True, stop=True)
            for gname in ("r", "hn", "in", "z"):
                nc.tensor.matmul(warm_ps[:, 0:1], bias_sb[gname][:],
                                 ones4_sb[:, 0:1], start=True, stop=True)

            for t in range(T):
                first = t == 0
                w = w1_sb if first else w_sb

                for g in range(GROUPS):
                    hg = h16[g]

                    def bias_mm(ps, gname, stop=False):
                        nc.tensor.matmul(ps[:], bias_sb[gname][:], ones4_sb[:],
                                         start=True, stop=stop,
                                         skip_group_check=True)

                    def mm_block(ps, g_idx, stop_last=True, after=None):
                        for tt in range(4):
                            for k in range(KT):
                                blk = ((g_idx * 4 + tt) * KT + k) * 128
                                mm = nc.tensor.matmul(
                                    ps[:, tt * b : (tt + 1) * b],
                                    w[:, blk : blk + 128],
                                    hg[:, k * b : (k + 1) * b],
                                    start=False,
                                    stop=(stop_last and tt == 3 and k == KT - 1),
                                    skip_group_check=True,
                                )
                                if after is not None:
                                    add_dep_helper(
                                        mm.ins, after.ins, sync=False,
                                        reason="z-block waits on rhn idMM",
                                    )
                                    after = None

                    r_ps = psum.tile([128, 4 * b], F32, tag=f"ps_r{g}")
                    hn_ps = psum.tile([128, 4 * b], F32, tag=f"ps_hn{g}")
                    in_ps = psum.tile([128, 4 * b], F32, tag=f"ps_in{g}")
                    z_ps = psum.tile([128, 4 * b], F32, tag=f"ps_z{g}")

                    # r block, then its sigmoid (hidden under hn/in stream)
                    bias_mm(r_ps, "r")
                    mm_block(r_ps, 0)
                    r16 = work.tile([128, 4 * b], F16, tag=f"r16_{g}")
                    nc.scalar.activation(r16[:], r_ps[:], sig)

                    # hn block, then rhn = r * hn (hidden under in stream)
                    bias_mm(hn_ps, "hn")
                    mm_block(hn_ps, 1)
                    rhn16 = work.tile([128, 4 * b], F16, tag=f"rhn_{g}")
                    nc.vector.tensor_mul(rhn16[:], r16[:], hn_ps[:])

                    # in block; then PE adds rhn into in_ps (identity matmul)
                    bias_mm(in_ps, "in")
                    if not first:
                        mm_block(in_ps, 2, stop_last=False)
                    id_mm = nc.tensor.matmul(in_ps[:], ident_sb[:], rhn16[:],
                                             start=False, stop=True,
                                             skip_group_check=True)
                    n16 = work.tile([128, 4 * b], F16, tag=f"n_{g}")
                    tanh_inst = nc.scalar.activation(n16[:], in_ps[:], tanh)
                    d16 = work.tile([128, 4 * b], F16, tag=f"d_{g}")
                    nc.vector.tensor_sub(d16[:], hg[:], n16[:])

                    # z block LAST (forced after the idMM); the tail
                    # (sigma_z, zd, h') hides under the other group's stream
                    bias_mm(z_ps, "z")
                    mm_block(z_ps, 2 if first else 3, after=id_mm)
                    z16 = work.tile([128, 4 * b], F16, tag=f"z16_{g}")
                    sigz_inst = nc.scalar.activation(z16[:], z_ps[:], sig)
                    add_dep_helper(sigz_inst.ins, tanh_inst.ins, sync=False,
                                   reason="sigma_z waits on tanh (ACT order)")
                    zd16 = work.tile([128, 4 * b], F16, tag=f"zd_{g}")
                    nc.vector.tensor_mul(zd16[:], z16[:], d16[:])
                    h16_new = state.tile([128, 4 * b], F16, tag=f"h16_{g}")
                    nc.vector.tensor_add(h16_new[:], zd16[:], n16[:])
                    nc.sync.dma_start(
                        out_d[t, :, g * 4 * b : (g + 1) * 4 * b], h16_new[:])
                    h16[g] = h16_new

    if not nc.is_finalized():
        nc.finalize()
    return nc


def _prep_host(encoder_outputs, W_ih, W_hh, b_ih, b_hh, T, n_cores, b_loc):
    """Shard + lay out host inputs; returns per-core in_maps."""
    W_ih = np.asarray(W_ih, dtype=np.float32)
    W_hh = np.asarray(W_hh, dtype=np.float32)
    b_ih = np.asarray(b_ih, dtype=np.float32)
    b_hh = np.asarray(b_hh, dtype=np.float32)
    enc = np.asarray(encoder_outputs, dtype=np.float32)
    b = b_loc // GROUPS

    W_r = W_ih[:H] + W_hh[:H]
    W_z = W_ih[H : 2 * H] + W_hh[H : 2 * H]
    W_hn = W_hh[2 * H :]
    W_in = W_ih[2 * H :]

    def blocks_of(*gates):
        cols = []
        for Wm in gates:
            WmT = np.ascontiguousarray(Wm.T)  # (512, 512)
            for tt in range(4):
                for k in range(KT):
                    cols.append(
                        WmT[128 * k : 128 * (k + 1), 128 * tt : 128 * (tt + 1)]
                    )
        return np.concatenate(cols, axis=1).astype(np.float16)

    wc_host = blocks_of(W_r, W_hn, W_in, W_z)            # (128, 64*128)
    w1_host = blocks_of(W_hh[:H], W_hh[2 * H :], W_hh[H : 2 * H])

    bst = np.concatenate([
        (b_ih[:H] + b_hh[:H]).reshape(4, 128),
        b_hh[2 * H :].reshape(4, 128),
        b_ih[2 * H :].reshape(4, 128),
        (b_ih[H : 2 * H] + b_hh[H : 2 * H]).reshape(4, 128),
    ], axis=0).astype(np.float16)  # (16, 128)
    ones4 = np.kron(np.eye(4, dtype=np.float16), np.ones((1, b), np.float16))
    ident = np.eye(128, dtype=np.float16)

    h0 = enc[0, :, -1, :]  # (128, 512)
    in_maps = []
    for c in range(n_cores):
        h0c = h0[c * b_loc : (c + 1) * b_loc]  # (b_loc, 512)
        # per group: (b, 512) -> transposed (128, KT*b); concat groups
        parts = []
        for g in range(GROUPS):
            hgc = h0c[g * b : (g + 1) * b]
            parts.append(
                hgc.reshape(b, KT, 128).transpose(2, 1, 0).reshape(128, KT * b)
            )
        h0t = np.ascontiguousarray(np.concatenate(parts, axis=1)).astype(
            np.float16)
        in_maps.append({
            "wc": wc_host, "w1": w1_host, "bst": bst,
            "ones4": ones4, "ident": ident, "h0t": h0t,
        })
    return in_maps


def _gather(results, T, n_cores, b_loc):
    b = b_loc // GROUPS
    out = np.empty((T, BATCH, H), dtype=np.float32)
    for c in range(n_cores):
        oc = results[c]["outT"]  # (T, 128, GROUPS*KT*b) fp16
        for g in range(GROUPS):
            og = oc[:, :, g * KT * b : (g + 1) * KT * b]
            rows = slice(c * b_loc + g * b, c * b_loc + (g + 1) * b)
            out[:, rows, :] = (
                og.reshape(T, 128, KT, b).transpose(0, 3, 2, 1)
                .reshape(T, b, H).astype(np.float32)
            )
    return out


_CACHE = {}


def kernel(encoder_outputs, W_ih, W_hh, b_ih, b_hh, out_len):
    T = int(out_len)
    assert T == T_STEPS, f"built for T={T_STEPS}, got {T}"
    key = (T, N_CORES)
    if key not in _CACHE:
        _CACHE[key] = _build(T)
    nc = _CACHE[key]

    in_maps = _prep_host(encoder_outputs, W_ih, W_hh, b_ih, b_hh,
                         T, N_CORES, B_LOC)
    res = run_bass_kernel_spmd(nc, in_maps, core_ids=list(range(N_CORES)))
    out = _gather(res.results, T, N_CORES, B_LOC)
    return out.reshape(T * BATCH, 1, H)


# revision 11
# speedup vs baseline: 1.2340x; 1.2340x over previous
"""Trainium2 Bass kernel for the GRU decoder (nn_Decoder_13168369730058).

Math (from the reference):
  h0 = encoder_outputs[0, :, -1, :]                       # (128, 512)
  step 1:   h1 = gru_cell(x=0, h0)
  step t>1: h_t = gru_cell(h_{t-1}, h_{t-1})   (carry is (h_new, h_new))

Because x == h from step 2 on, the two GRU matmuls fuse per gate:
  r  = sigmoid(Wr h + br)          Wr = Wih_r + Whh_r,  br = bih_r + bhh_r
  z  = sigmoid(Wz h + bz)          Wz = Wih_z + Whh_z,  bz = bih_z + bhh_z
  n  = tanh(Win h + bin + r * (Whn h + bhn))
  h' = n + z * (h - n)
Step 1 is the same with Wr,Wz -> Whh_{r,z} and no Win matmul (x = 0).

Distribution: data-parallel over batch, 16 rows per core on 8 cores, weights
replicated; the out_len recurrence is local to each core.

Layout: fully transposed on chip (H on partitions, batch on free dim). Each
128x128 weight block is the stationary operand and the transposed hidden
state h^T (128, 16) the moving operand, producing gate pre-activations in
PSUM. LDW+MM pairs sustain ~27ns (fp16 fast-weight-load); the r/z/hn gates
are stored fp8-e4m3 (per-gate global scales) to double their weight-load
rate where the hardware allows. Scale folding costs zero extra ops:
  - r/z dequant scales fold into the sigmoid's scale immediate
    (sigmoid(s * psum)), biases are pre-divided by s on the host;
  - hn's scale folds into the identity matmul that injects r*hn into the
    in-gate PSUM (identity replaced by s_hn * I).
The in-gate stays fp16 (the n-path is the error-sensitive one); measured
end-to-end rel err 8.9e-3 vs the fp32 reference (gate 2e-2).

Step schedule (PE order): [bias_r][r x16][bias_hn][hn x16][bias_in][in x16]
[idMM][bias_z][z x16]; sigma_r, r*hn, tanh run in the shadow of later
blocks; the z-gate is last so only sigma_z -> z*(h-n) -> +n is exposed.
add_dep_helper pins the z-block after the idMM and sigma_z after tanh
(the scheduler otherwise serializes the tail). State, tail tensors, and
the DMA'd output are all fp16.
"""

import os
import numpy as np
import ml_dtypes

import concourse.bacc as bacc
import concourse.mybir as mybir
import concourse.tile as tile
from concourse.tile import add_dep_helper
from concourse.bass_utils import run_bass_kernel_spmd

H = 512
BATCH = 128
N_CORES = int(os.environ.get("GRU_N_CORES", "8"))
T_STEPS = int(os.environ.get("GRU_T_STEPS", "1024"))
B_LOC = BATCH // N_CORES  # local batch per core (16)
KT = H // 128             # 4 k-tiles

F32 = mybir.dt.float32
F16 = mybir.dt.float16
F8 = mybir.dt.float8e4


def _build(T: int, b: int, scales):
    """Build the Bass program: T steps, b batch rows per core."""
    s_r, s_z, s_hn = scales
    nc = bacc.Bacc()

    # fp8 blocks: [r x16, hn x16, z x16]; fp16 blocks: [in x16]
    wc8_d = nc.dram_tensor("wc8", [128, 48 * 128], F8, kind="ExternalInput")
    wc16_d = nc.dram_tensor("wc16", [128, 16 * 128], F16, kind="ExternalInput")
    # w1 blocks: [r x16, hn x16, z x16] (step 1, weights = W_hh, fp16)
    w1_d = nc.dram_tensor("w1", [128, 48 * 128], F16, kind="ExternalInput")
    # bias stationaries: rows 0-3 b_r/s_r, 4-7 b_hn/s_hn, 8-11 b_in,
    # 12-15 b_z/s_z, 16-19 b_r, 20-23 b_hn, 24-27 b_z (unscaled, step 1)
    bst_d = nc.dram_tensor("bst", [28, 128], F16, kind="ExternalInput")
    ones4_d = nc.dram_tensor("ones4", [4, 4 * b], F16, kind="ExternalInput")
    # identities: cols 0:128 = I (step 1), 128:256 = s_hn * I
    ident_d = nc.dram_tensor("ident", [128, 256], F16, kind="ExternalInput")
    h0_d = nc.dram_tensor("h0t", [128, 4 * b], F16, kind="ExternalInput")
    out_d = nc.dram_tensor("outT", [T, 128, 4 * b], F16, kind="ExternalOutput")

    sig = mybir.ActivationFunctionType.Sigmoid
    tanh = mybir.ActivationFunctionType.Tanh

    with tile.TileContext(nc) as tc:
        with (
            tc.tile_pool(name="singles", bufs=1) as singles,
            tc.tile_pool(name="state", bufs=2) as state,
            tc.tile_pool(name="work", bufs=2) as work,
            tc.tile_pool(name="psum", bufs=2, space="PSUM") as psum,
        ):
            w8_sb = singles.tile([128, 48 * 128], F8)
            nc.sync.dma_start(w8_sb[:], wc8_d[:])
            w16_sb = singles.tile([128, 16 * 128], F16)
            nc.sync.dma_start(w16_sb[:], wc16_d[:])
            w1_sb = singles.tile([128, 48 * 128], F16)
            nc.sync.dma_start(w1_sb[:], w1_d[:])
            bias_sb = {}
            for gi, gname in enumerate(
                    ("r", "hn", "in", "z", "r1", "hn1", "z1")):
                t_ = singles.tile([4, 128], F16, tag=f"b{gname}")
                nc.sync.dma_start(t_[:], bst_d[4 * gi : 4 * (gi + 1)])
                bias_sb[gname] = t_
            ones4_sb = singles.tile([4, 4 * b], F16)
            nc.sync.dma_start(ones4_sb[:], ones4_d[:])
            ident_sb = singles.tile([128, 256], F16)
            nc.sync.dma_start(ident_sb[:], ident_d[:])

            h16 = state.tile([128, 4 * b], F16, tag="h16")
            nc.sync.dma_start(h16[:], h0_d[:])

            # Warm-up: hardware allows ONE embedded sync wait per instruction;
            # have the PE observe every init DMA here so loop matmuls carry a
            # single cross-engine wait (on h16 only).
            warm_ps = psum.tile([128, 8], F32, tag="r")
            nc.tensor.matmul(warm_ps[:, 0:8], w8_sb[:, 0:128], w1_sb[:, 0:8],
                             start=True, stop=True)
            nc.tensor.matmul(warm_ps[:, 0:8], w1_sb[:, 0:128], w1_sb[:, 0:8],
                             start=True, stop=True)
            nc.tensor.matmul(warm_ps[:, 0:8], w16_sb[:, 0:128], w16_sb[:, 0:8],
                             start=True, stop=True)
            nc.tensor.matmul(warm_ps[:, 0:8], ident_sb[:, 0:128],
                             w1_sb[:, 0:8], start=True, stop=True)
            for gname in ("r", "hn", "in", "z", "r1", "hn1", "z1"):
                nc.tensor.matmul(warm_ps[:, 0:1], bias_sb[gname][:],
                                 ones4_sb[:, 0:1], start=True, stop=True)

            for t in range(T):
                first = t == 0

                r_ps = psum.tile([128, 4 * b], F32, tag="r")
                hn_ps = psum.tile([128, 4 * b], F32, tag="hn")
                in_ps = psum.tile([128, 4 * b], F32, tag="in")
                z_ps = psum.tile([128, 4 * b], F32, tag="z")

                def bias_mm(ps, gname, stop=False):
                    nc.tensor.matmul(ps[:], bias_sb[gname][:], ones4_sb[:],
                                     start=True, stop=stop,
                                     skip_group_check=True)

                def mm_block(ps, wtile, g_idx, stop_last=True, after=None):
                    for tt in range(4):
                        for k in range(KT):
                            blk = ((g_idx * 4 + tt) * KT + k) * 128
                            mm = nc.tensor.matmul(
                                ps[:, tt * b : (tt + 1) * b],
                                wtile[:, blk : blk + 128],
                                h16[:, k * b : (k + 1) * b],
                                start=False,
                                stop=(stop_last and tt == 3 and k == KT - 1),
                                skip_group_check=True,
                            )
                            if after is not None:
                                add_dep_helper(
                                    mm.ins, after.ins, sync=False,
                                    reason="z-block waits on rhn idMM")
                                after = None

                # r block, then its sigmoid (hidden under hn/in stream)
                bias_mm(r_ps, "r1" if first else "r")
                mm_block(r_ps, w1_sb if first else w8_sb, 0)
                r16 = work.tile([128, 4 * b], F16, tag="r16")
                nc.scalar.activation(r16[:], r_ps[:], sig,
                                     scale=1.0 if first else s_r)

                # hn block, then rhn = r * hn (hidden under in stream)
                bias_mm(hn_ps, "hn1" if first else "hn")
                mm_block(hn_ps, w1_sb if first else w8_sb, 1)
                rhn16 = work.tile([128, 4 * b], F16, tag="rhn")
                nc.vector.tensor_mul(rhn16[:], r16[:], hn_ps[:])

                # in block; then PE adds s_hn * rhn into in_ps via the
                # (scaled) identity matmul
                bias_mm(in_ps, "in")
                if not first:
                    mm_block(in_ps, w16_sb, 0, stop_last=False)
                idcols = ident_sb[:, 0:128] if first else ident_sb[:, 128:256]
                id_mm = nc.tensor.matmul(in_ps[:], idcols, rhn16[:],
                                         start=False, stop=True,
                                         skip_group_check=True)
                n16 = work.tile([128, 4 * b], F16, tag="n")
                tanh_inst = nc.scalar.activation(n16[:], in_ps[:], tanh)
                d16 = work.tile([128, 4 * b], F16, tag="d")
                nc.vector.tensor_sub(d16[:], h16[:], n16[:])

                # z block LAST (forced after the idMM so tanh's input is
                # ready mid-stream): exposed tail = sigmoid + 2 fp16 DVE ops
                bias_mm(z_ps, "z1" if first else "z")
                mm_block(z_ps, w1_sb if first else w8_sb, 2, after=id_mm)
                z16 = work.tile([128, 4 * b], F16, tag="z16")
                sigz_inst = nc.scalar.activation(z16[:], z_ps[:], sig,
                                                 scale=1.0 if first else s_z)
                add_dep_helper(sigz_inst.ins, tanh_inst.ins, sync=False,
                               reason="sigma_z waits on tanh (ACT order)")
                zd16 = work.tile([128, 4 * b], F16, tag="zd")
                nc.vector.tensor_mul(zd16[:], z16[:], d16[:])
                h16_new = state.tile([128, 4 * b], F16, tag="h16")
                nc.vector.tensor_add(h16_new[:], zd16[:], n16[:])
                nc.sync.dma_start(out_d[t], h16_new[:])
                h16 = h16_new

    if not nc.is_finalized():
        nc.finalize()
    return nc


def _quant_scales(W_ih, W_hh):
    W_r = W_ih[:H] + W_hh[:H]
    W_z = W_ih[H : 2 * H] + W_hh[H : 2 * H]
    W_hn = W_hh[2 * H :]
    s_r = float(np.float16(np.max(np.abs(W_r)) / 240.0))
    s_z = float(np.float16(np.max(np.abs(W_z)) / 240.0))
    s_hn = float(np.float16(np.max(np.abs(W_hn)) / 240.0))
    return (s_r, s_z, s_hn), (W_r, W_z, W_hn)


def _prep_host(encoder_outputs, W_ih, W_hh, b_ih, b_hh, T, n_cores, b):
    """Shard + lay out host inputs; returns per-core in_maps."""
    W_ih = np.asarray(W_ih, dtype=np.float32)
    W_hh = np.asarray(W_hh, dtype=np.float32)
    b_ih = np.asarray(b_ih, dtype=np.float32)
    b_hh = np.asarray(b_hh, dtype=np.float32)
    enc = np.asarray(encoder_outputs, dtype=np.float32)

    (s_r, s_z, s_hn), (W_r, W_z, W_hn) = _quant_scales(W_ih, W_hh)
    W_in = W_ih[2 * H :]

    def blocks_of(dt, *gates):
        cols = []
        for Wm in gates:
            WmT = np.ascontiguousarray(Wm.T)  # (512, 512)
            for tt in range(4):
                for k in range(KT):
                    cols.append(
                        WmT[128 * k : 128 * (k + 1), 128 * tt : 128 * (tt + 1)]
                    )
        return np.concatenate(cols, axis=1).astype(dt)

    def q8(W, s):
        return np.clip(W / s, -240, 240)

    wc8 = blocks_of(ml_dtypes.float8_e4m3,
                    q8(W_r, s_r), q8(W_hn, s_hn), q8(W_z, s_z))
    wc16 = blocks_of(np.float16, W_in)
    w1 = blocks_of(np.float16, W_hh[:H], W_hh[2 * H :], W_hh[H : 2 * H])

    b_r = b_ih[:H] + b_hh[:H]
    b_z = b_ih[H : 2 * H] + b_hh[H : 2 * H]
    b_hn = b_hh[2 * H :]
    b_in = b_ih[2 * H :]
    bst = np.concatenate([
        (b_r / s_r).reshape(4, 128), (b_hn / s_hn).reshape(4, 128),
        b_in.reshape(4, 128), (b_z / s_z).reshape(4, 128),
        b_r.reshape(4, 128), b_hn.reshape(4, 128), b_z.reshape(4, 128),
    ], axis=0).astype(np.float16)  # (28, 128)
    ones4 = np.kron(np.eye(4, dtype=np.float16), np.ones((1, b), np.float16))
    ident = np.concatenate(
        [np.eye(128, dtype=np.float16),
         (np.eye(128) * np.float16(s_hn)).astype(np.float16)], axis=1)

    h0 = enc[0, :, -1, :]  # (128, 512)
    in_maps = []
    for c in range(n_cores):
        h0c = h0[c * b : (c + 1) * b]  # (b, 512)
        h0t = np.ascontiguousarray(
            h0c.reshape(b, KT, 128).transpose(2, 1, 0).reshape(128, KT * b)
        ).astype(np.float16)
        in_maps.append({
            "wc8": wc8, "wc16": wc16, "w1": w1, "bst": bst,
            "ones4": ones4, "ident": ident, "h0t": h0t,
        })
    return in_maps


def _gather(results, T, n_cores, b):
    out = np.empty((T, BATCH, H), dtype=np.float32)
    for c in range(n_cores):
        oc = results[c]["outT"]  # (T, 128, KT*b) fp16, free = [k][j]
        out[:, c * b : (c + 1) * b, :] = (
            oc.reshape(T, 128, KT, b).transpose(0, 3, 2, 1).reshape(T, b, H)
            .astype(np.float32)
        )
    return out


_CACHE = {}


def kernel(encoder_outputs, W_ih, W_hh, b_ih, b_hh, out_len):
    T = int(out_len)
    assert T == T_STEPS, f"built for T={T_STEPS}, got {T}"
    W_ih_np = np.asarray(W_ih, dtype=np.float32)
    W_hh_np = np.asarray(W_hh, dtype=np.float32)
    scales, _ = _quant_scales(W_ih_np, W_hh_np)
    key = (T, N_CORES, scales)
    if key not in _CACHE:
        _CACHE[key] = _build(T, B_LOC, scales)
    nc = _CACHE[key]

    in_maps = _prep_host(encoder_outputs, W_ih_np, W_hh_np, b_ih, b_hh,
                         T, N_CORES, B_LOC)
    res = run_bass_kernel_spmd(nc, in_maps, core_ids=list(range(N_CORES)))
    out = _gather(res.results, T, N_CORES, B_LOC)
    return out.reshape(T * BATCH, 1, H)


# revision 14
# speedup vs baseline: 1.4773x; 1.1971x over previous
"""Trainium2 Bass kernel for the GRU decoder (nn_Decoder_13168369730058).

Math (from the reference):
  h0 = encoder_outputs[0, :, -1, :]                       # (128, 512)
  step 1:   h1 = gru_cell(x=0, h0)
  step t>1: h_t = gru_cell(h_{t-1}, h_{t-1})   (carry is (h_new, h_new))

Because x == h from step 2 on, the two GRU matmuls fuse per gate:
  r  = sigmoid(Wr h + br)          Wr = Wih_r + Whh_r,  br = bih_r + bhh_r
  z  = sigmoid(Wz h + bz)          Wz = Wih_z + Whh_z,  bz = bih_z + bhh_z
  n  = tanh(Win h + bin + r * (Whn h + bhn))
  h' = n + z * (h - n)
Step 1 is the same with Wr,Wz -> Whh_{r,z} and no Win matmul (x = 0).

Distribution: data-parallel over batch, 16 rows per core on 8 cores, weights
replicated; the out_len recurrence is local to each core.

Layout: fully transposed on chip (H on partitions, batch on free dim). Each
128x128 fp16 weight block is the stationary operand (fast-weight-load;
LDW+MM pairs sustain ~27ns in a dense stream) and the transposed hidden
state h^T (128, 16) the moving operand, producing gate pre-activations in
PSUM directly. fp8 weights were measured SLOWER per pair (30ns) - FWL on
this silicon is already at its 32-bit/partition/cycle ceiling with fp16.

Step schedule (PE order):
  [bias_r][r x16][bias_hn][hn x16][bias_in][in x16][idMM][bias_za]
  [bias_zb][z_a x8][z_b x8]
  - biases are seeded into each gate's PSUM by matmuls whose stationaries
    are zero-padded to 128 partitions: a 4-partition stationary disables
    fast-weight-load and stalls the PE pipeline ~200ns per load;
  - r*hn is injected into the in-gate PSUM by an identity matmul so tanh
    reads the complete pre-activation from PSUM (no DVE add);
  - the z-gate runs LAST, split into two PSUM tiles (separate banks) so
    sigmoid_za overlaps the z_b matmuls and the final h' is produced in
    two halves, releasing the next step's k0/k1 matmuls early.
add_dep_helper pins the z-blocks after the idMM and the z-sigmoids after
tanh (the scheduler otherwise serializes the tail). State, tail tensors,
and the DMA'd output are all fp16 (measured 6.2e-4 rel err end-to-end;
the trajectory is contractive, not chaotic).
"""

import os
import numpy as np

import concourse.bacc as bacc
import concourse.mybir as mybir
import concourse.tile as tile
from concourse.tile import add_dep_helper
from concourse.bass_utils import run_bass_kernel_spmd

H = 512
BATCH = 128
N_CORES = int(os.environ.get("GRU_N_CORES", "8"))
T_STEPS = int(os.environ.get("GRU_T_STEPS", "1024"))
B_LOC = BATCH // N_CORES  # local batch per core (16)
KT = H // 128             # 4 k-tiles

F32 = mybir.dt.float32
F16 = mybir.dt.float16


def _build(T: int, b: int):
    """Build the Bass program: T steps, b batch rows per core."""
    nc = bacc.Bacc()

    # wc blocks: [r x16, hn x16, in x16, z x16]; each gate m-tile-major, then k
    wc_d = nc.dram_tensor("wc", [128, 64 * 128], F16, kind="ExternalInput")
    # w1 blocks: [r x16, hn x16, z x16] (step 1, weights = W_hh only)
    w1_d = nc.dram_tensor("w1", [128, 48 * 128], F16, kind="ExternalInput")
    # bias stationaries, zero-padded to 128 partitions: blocks [r|hn|in|z],
    # rows 0-3 of each block hold bias[128k : 128(k+1)] for m-tile k
    bst_d = nc.dram_tensor("bst", [128, 4 * 128], F16, kind="ExternalInput")
    # ones: rows 0-3 carry the m-tile selector pattern, rows 4-127 zero
    ones_d = nc.dram_tensor("ones", [128, 4 * b], F16, kind="ExternalInput")
    ident_d = nc.dram_tensor("ident", [128, 128], F16, kind="ExternalInput")
    h0_d = nc.dram_tensor("h0t", [128, 4 * b], F16, kind="ExternalInput")
    out_d = nc.dram_tensor("outT", [T, 128, 4 * b], F16, kind="ExternalOutput")

    sig = mybir.ActivationFunctionType.Sigmoid
    tanh = mybir.ActivationFunctionType.Tanh

    with tile.TileContext(nc) as tc:
        with (
            tc.tile_pool(name="singles", bufs=1) as singles,
            tc.tile_pool(name="state", bufs=2) as state,
            tc.tile_pool(name="work", bufs=2) as work,
            tc.tile_pool(name="psum", bufs=2, space="PSUM") as psum,
            tc.tile_pool(name="psumz", bufs=1, space="PSUM") as psumz,
        ):
            w_sb = singles.tile([128, 64 * 128], F16)
            nc.sync.dma_start(w_sb[:], wc_d[:])
            w1_sb = singles.tile([128, 48 * 128], F16)
            nc.sync.dma_start(w1_sb[:], w1_d[:])
            bias_sb = {}
            for gi, gname in enumerate(("r", "hn", "in", "z")):
                t_ = singles.tile([128, 128], F16, tag=f"b{gname}")
                nc.sync.dma_start(t_[:], bst_d[:, 128 * gi : 128 * (gi + 1)])
                bias_sb[gname] = t_
            ones_sb = singles.tile([128, 4 * b], F16)
            nc.sync.dma_start(ones_sb[:], ones_d[:])
            ident_sb = singles.tile([128, 128], F16)
            nc.sync.dma_start(ident_sb[:], ident_d[:])

            h16 = state.tile([128, 4 * b], F16, tag="h16")
            nc.sync.dma_start(h16[:], h0_d[:])

            # Warm-up: hardware allows ONE embedded sync wait per instruction;
            # have the PE observe every init DMA here so loop matmuls carry a
            # single cross-engine wait (on h16 only).
            warm_ps = psum.tile([128, 8], F32, tag="r")
            nc.tensor.matmul(warm_ps[:, 0:8], w_sb[:, 0:128], w_sb[:, 0:8],
                             start=True, stop=True)
            nc.tensor.matmul(warm_ps[:, 0:8], w1_sb[:, 0:128], w1_sb[:, 0:8],
                             start=True, stop=True)
            nc.tensor.matmul(warm_ps[:, 0:8], ident_sb[:], w_sb[:, 0:8],
                             start=True, stop=True)
            for gname in ("r", "hn", "in", "z"):
                nc.tensor.matmul(warm_ps[:, 0:8], bias_sb[gname][:],
                                 ones_sb[:, 0:8], start=True, stop=True)

            for t in range(T):
                first = t == 0
                w = w1_sb if first else w_sb

                r_ps = psum.tile([128, 4 * b], F32, tag="r")
                hn_ps = psum.tile([128, 4 * b], F32, tag="hn")
                in_ps = psum.tile([128, 4 * b], F32, tag="in")
                za_ps = psumz.tile([128, 2 * b], F32, tag="za")
                zb_ps = psumz.tile([128, 2 * b], F32, tag="zb")

                def mm_block(ps, g_idx, tiles=(0, 1, 2, 3), col0=0,
                             stop_last=True, after=None):
                    for ti, tt in enumerate(tiles):
                        for k in range(KT):
                            blk = ((g_idx * 4 + tt) * KT + k) * 128
                            mm = nc.tensor.matmul(
                                ps[:, (col0 + ti) * b : (col0 + ti + 1) * b],
                                w[:, blk : blk + 128],
                                h16[:, k * b : (k + 1) * b],
                                start=False,
                                stop=(stop_last and ti == len(tiles) - 1
                                      and k == KT - 1),
                                skip_group_check=True,
                            )
                            if after is not None:
                                add_dep_helper(
                                    mm.ins, after.ins, sync=False,
                                    reason="z-block waits on rhn idMM")
                                after = None

                # r block, then its sigmoid (hidden under hn/in stream)
                nc.tensor.matmul(r_ps[:], bias_sb["r"][:], ones_sb[:],
                                 start=True, stop=False, skip_group_check=True)
                mm_block(r_ps, 0)
                r16 = work.tile([128, 4 * b], F16, tag="r16")
                nc.scalar.activation(r16[:], r_ps[:], sig)

                # hn block, then rhn = r * hn (hidden under in stream)
                nc.tensor.matmul(hn_ps[:], bias_sb["hn"][:], ones_sb[:],
                                 start=True, stop=False, skip_group_check=True)
                mm_block(hn_ps, 1)
                rhn16 = work.tile([128, 4 * b], F16, tag="rhn")
                nc.vector.tensor_mul(rhn16[:], r16[:], hn_ps[:])

                # in block; then PE adds rhn into in_ps via identity matmul
                nc.tensor.matmul(in_ps[:], bias_sb["in"][:], ones_sb[:],
                                 start=True, stop=first, skip_group_check=True)
                if not first:
                    mm_block(in_ps, 2, stop_last=False)
                id_mm = nc.tensor.matmul(in_ps[:], ident_sb[:], rhn16[:],
                                         start=False, stop=True,
                                         skip_group_check=True)
                n16 = work.tile([128, 4 * b], F16, tag="n")
                tanh_inst = nc.scalar.activation(n16[:], in_ps[:], tanh)
                d16 = work.tile([128, 4 * b], F16, tag="d")
                nc.vector.tensor_sub(d16[:], h16[:], n16[:])

                # z blocks LAST (forced after the idMM), split into two PSUM
                # banks so sigmoid_za overlaps the z_b matmuls; exposed tail
                # per half = sigmoid + 2 fp16 DVE ops
                zg = 2 if first else 3
                nc.tensor.matmul(za_ps[:], bias_sb["z"][:], ones_sb[:, 0:2*b],
                                 start=True, stop=False, skip_group_check=True)
                nc.tensor.matmul(zb_ps[:], bias_sb["z"][:], ones_sb[:, 2*b:4*b],
                                 start=True, stop=False, skip_group_check=True)
                mm_block(za_ps, zg, tiles=(0, 1), after=id_mm)
                mm_block(zb_ps, zg, tiles=(2, 3))
                za16 = work.tile([128, 2 * b], F16, tag="za16")
                siga = nc.scalar.activation(za16[:], za_ps[:], sig)
                add_dep_helper(siga.ins, tanh_inst.ins, sync=False,
                               reason="sigma_za waits on tanh (ACT order)")
                zb16 = work.tile([128, 2 * b], F16, tag="zb16")
                sigb = nc.scalar.activation(zb16[:], zb_ps[:], sig)
                add_dep_helper(sigb.ins, siga.ins, sync=False,
                               reason="sigma_zb waits on sigma_za (ACT order)")

                h16_new = state.tile([128, 4 * b], F16, tag="h16")
                zda = work.tile([128, 2 * b], F16, tag="zda")
                nc.vector.tensor_mul(zda[:], za16[:], d16[:, 0 : 2 * b])
                ha = nc.vector.tensor_add(h16_new[:, 0 : 2 * b], zda[:],
                                          n16[:, 0 : 2 * b])
                zdb = work.tile([128, 2 * b], F16, tag="zdb")
                zdb_mm = nc.vector.tensor_mul(zdb[:], zb16[:],
                                              d16[:, 2 * b : 4 * b])
                add_dep_helper(zdb_mm.ins, ha.ins, sync=False,
                               reason="zdb after h'_a (DVE order, early k0/k1)")
                nc.vector.tensor_add(h16_new[:, 2 * b : 4 * b], zdb[:],
                                     n16[:, 2 * b : 4 * b])
                nc.sync.dma_start(out_d[t], h16_new[:])
                h16 = h16_new

    if not nc.is_finalized():
        nc.finalize()
    return nc


def _prep_host(encoder_outputs, W_ih, W_hh, b_ih, b_hh, T, n_cores, b):
    """Shard + lay out host inputs; returns per-core in_maps."""
    W_ih = np.asarray(W_ih, dtype=np.float32)
    W_hh = np.asarray(W_hh, dtype=np.float32)
    b_ih = np.asarray(b_ih, dtype=np.float32)
    b_hh = np.asarray(b_hh, dtype=np.float32)
    enc = np.asarray(encoder_outputs, dtype=np.float32)

    W_r = W_ih[:H] + W_hh[:H]
    W_z = W_ih[H : 2 * H] + W_hh[H : 2 * H]
    W_hn = W_hh[2 * H :]
    W_in = W_ih[2 * H :]

    def blocks_of(*gates):
        cols = []
        for Wm in gates:
            WmT = np.ascontiguousarray(Wm.T)  # (512, 512)
            for tt in range(4):
                for k in range(KT):
                    cols.append(
                        WmT[128 * k : 128 * (k + 1), 128 * tt : 128 * (tt + 1)]
                    )
        return np.concatenate(cols, axis=1).astype(np.float16)

    wc_host = blocks_of(W_r, W_hn, W_in, W_z)            # (128, 64*128)
    w1_host = blocks_of(W_hh[:H], W_hh[2 * H :], W_hh[H : 2 * H])

    def bias128(bvec):
        m = np.zeros((128, 128), np.float16)
        m[0:4, :] = bvec.reshape(4, 128).astype(np.float16)
        return m

    bst = np.concatenate([
        bias128(b_ih[:H] + b_hh[:H]),
        bias128(b_hh[2 * H :]),
        bias128(b_ih[2 * H :]),
        bias128(b_ih[H : 2 * H] + b_hh[H : 2 * H]),
    ], axis=1)  # (128, 512)
    ones = np.zeros((128, 4 * b), np.float16)
    ones[0:4] = np.kron(np.eye(4, dtype=np.float16),
                        np.ones((1, b), np.float16))
    ident = np.eye(128, dtype=np.float16)

    h0 = enc[0, :, -1, :]  # (128, 512)
    in_maps = []
    for c in range(n_cores):
        h0c = h0[c * b : (c + 1) * b]  # (b, 512)
        h0t = np.ascontiguousarray(
            h0c.reshape(b, KT, 128).transpose(2, 1, 0).reshape(128, KT * b)
        ).astype(np.float16)
        in_maps.append({
            "wc": wc_host, "w1": w1_host, "bst": bst,
            "ones": ones, "ident": ident, "h0t": h0t,
        })
    return in_maps


def _gather(results, T, n_cores, b):
    out = np.empty((T, BATCH, H), dtype=np.float32)
    for c in range(n_cores):
        oc = results[c]["outT"]  # (T, 128, KT*b) fp16, free = [k][j]
        out[:, c * b : (c + 1) * b, :] = (
            oc.reshape(T, 128, KT, b).transpose(0, 3, 2, 1).reshape(T, b, H)
            .astype(np.float32)
        )
    return out


_CACHE = {}


def kernel(encoder_outputs, W_ih, W_hh, b_ih, b_hh, out_len):
    T = int(out_len)
    assert T == T_STEPS, f"built for T={T_STEPS}, got {T}"
    key = (T, N_CORES)
    if key not in _CACHE:
        _CACHE[key] = _build(T, B_LOC)
    nc = _CACHE[key]

    in_maps = _prep_host(encoder_outputs, W_ih, W_hh, b_ih, b_hh,
                         T, N_CORES, B_LOC)
    res = run_bass_kernel_spmd(nc, in_maps, core_ids=list(range(N_CORES)))
    out = _gather(res.results, T, N_CORES, B_LOC)
    return out.reshape(T * BATCH, 1, H)
